# revision 41
# baseline (speedup 1.0000x reference)
"""Trainium2 Bass kernel for nn_AttenCross (sparse_attention).

reference:
    scores = einsum('bqd,bkd->bqk', Q, D) / sqrt(H)
    scores = where(doc_mask==0, -9999, scores)
    attn   = softmax(scores, -1)
    out    = sum over k of (attn * sim), then sum over q -> (B, 1)

Strategy (8 cores, data-parallel over batch, 2 batches/core), v2:

Host-side prep (sharding/layout/encoding only, exact for any inputs):
~50% of doc positions are masked (doc_mask ~ Bernoulli(0.5)), and masked
columns contribute exactly nothing once handled by counting, so the doc
axis is COMPACTED host-side: keep only unmasked doc columns (of D and
sim), zero-pad to K_pad = ceil(max_b keff[b]/128)*128.  A padded column
of D is all-zero => its score is exactly 0 => exp is exactly 1, so
subtracting the pad count from the exp row-sum reproduces the exact
softmax denominator; padded sim columns are zero so they add nothing to
the numerator.  (No row-max subtraction: scores ~ N(0,1); softmax is
shift-invariant.)  All tensors are converted to fp16 host-side: PE runs
fp16 at full rate (1 col/cycle vs ~2.2 for f32r), DMA bytes halve, and
DVE 16-bit ops run in 2x/4x perf modes; fp16's 10-bit mantissa keeps the
overall rel-err ~1e-3.

Device, per batch (per q-tile of 128 queries), ACT(exp)-paced pipeline:
  - PE: QK^T fp16 matmuls into PSUM chunks (1024 double-buffered +
    K_pad-1024 single-buffered; PSUM is 8 banks: 4 + 3 + 1 for the
    column-sum accumulator -- a full-width double-buffered score tile
    would not fit, which is what forces 2 ACTIVATEs per q-tile).
  - ACT: E = exp(scale*psum) -> fp16 SBUF, one ACTIVATE per chunk, with
    accum_out producing the per-chunk row-sums (den parts) for free-ish
    (the DVE alternatives -- tensor_scalar-with-accum aka
    TENSOR_SCALAR_CACHE_REDUCE, and tensor_reduce -- both run at 1x
    ~2.5us per q-tile on HW; ACT's ACTIVATION_READ_ACCUMULATOR is 311ns;
    the ACT accumulator resets per ACTIVATE so each chunk needs its own).
  - DVE: den-parts sum + cnt subtract + reciprocal -> w into column 0 of
    a rotating fp16 [128,128] tile; P = E * sim fp16 (2x mode), split at
    the chunk boundary so it starts right after chunk A's exp.
  - PE: column-sum matmuls with w as the stationary operand accumulate
    sum_q P[q,k]/den_q into a per-batch [128,512] PSUM bank; row 0 of
    that bank reduced (DVE) to the scalar batch output at the end.
    These trail the scores pipeline by one q-tile (software pipelining)
    so the next q-tile's QK matmuls issue ahead of them on the PE queue.
  - DMA: sim streams alternate between the sync HWDGE queue and the
    GpSimd SWDGE queue; per-partition runs are kept >= 2KB (narrow
    column slices degrade into tiny descriptors that throttle the
    queue); no HAM warm-up (cold PE hides behind the ACT pacing).
Output per core: [1, BPC] fp32; host stacks to [16, 1] fp32.

Measured on the target HW: ~68-70us (vs 138us for the f32r/full-K
baseline); exp streaming is the bound: 16 q-tiles x (2128 elems + 2
instruction bubbles + 2 accumulator reads)/1.2GHz ~= 44us, plus ~7us
framework preamble, ~4us fill, ~4us drain chain, ~9.5us teardown.
"""

import math

import numpy as np

import concourse.bacc as bacc
import concourse.tile as tile
import concourse.mybir as mybir
from concourse.bass_utils import run_bass_kernel_spmd

B, QL, DL, H = 16, 1024, 4096, 128
NCORES = 8
BPC = B // NCORES  # batches per core
QT_N = QL // 128  # 8 q-tiles per batch
SCALE = 1.0 / float(np.sqrt(H))

f32 = mybir.dt.float32
f16 = mybir.dt.float16

_CACHED = {}


def _plan_chunks(k_pad):
    """Split the doc axis into PSUM-resident chunks: (offset_lo, offset_hi,
    tag, bufs).

    Expected path (k_pad <= 2560): chunk A [128,1024] double-buffered (4
    banks) + chunk B [128, k_pad-1024 <= 1536] single-buffered (<=3
    banks) + 1 bank for the column-sum accumulator = 8.  Generic
    fallback for larger k_pad: 1024-wide chunks cycling three
    single-buffered tags."""
    if k_pad <= 1024:
        return [(0, k_pad, "A", 2)]
    if k_pad <= 2560:
        return [(0, 1024, "A", 2), (1024, k_pad, "B", 1)]
    chunks = []
    off = 0
    i = 0
    while off < k_pad:
        w = min(1024, k_pad - off)
        chunks.append((off, off + w, "BCD"[i % 3], 1))
        off += w
        i += 1
    return chunks


def _build(k_pad):
    chunks = _plan_chunks(k_pad)

    nc = bacc.Bacc("TRN2", target_bir_lowering=False, debug=False)

    qtd = nc.dram_tensor("qt", [BPC, H, QL], f16, kind="ExternalInput")
    dtd = nc.dram_tensor("dt", [BPC, H, k_pad], f16, kind="ExternalInput")
    sd = nc.dram_tensor("s", [BPC, QL, k_pad], f16, kind="ExternalInput")
    ckd = nc.dram_tensor("ck", [BPC, 1], f32, kind="ExternalInput")
    outd = nc.dram_tensor("o", [1, BPC], f32, kind="ExternalOutput")

    with tile.TileContext(nc) as tc:
        with (
            tc.tile_pool(name="const", bufs=1) as const,
            tc.tile_pool(name="qtp", bufs=2) as qtp,
            tc.tile_pool(name="dtp", bufs=2) as dtp,
            tc.tile_pool(name="simp", bufs=4) as simp,
            tc.tile_pool(name="ep", bufs=3) as ep,
            tc.tile_pool(name="pp", bufs=3) as pp,
            tc.tile_pool(name="small", bufs=4) as small,
            tc.tile_pool(name="bsm", bufs=2) as bsm,
            tc.tile_pool(name="outp", bufs=1) as outp,
            tc.tile_pool(name="ps", bufs=1, space="PSUM") as psp,
            tc.tile_pool(name="pacc", bufs=1, space="PSUM") as pacc,
        ):
            # stationary w-tiles for the column-sum matmuls: col 0 = 1/den,
            # cols 1..127 stay zero forever (three rotating, since the
            # column-sums trail the scores pipeline by two steps)
            r128s = []
            for ri in range(3):
                r = const.tile([128, 128], f16, tag=f"r128_{ri}", name=f"r128_{ri}")
                nc.vector.memset(r, 0.0)
                r128s.append(r)

            outsb = outp.tile([1, BPC], f32, tag="outsb")

            import concourse.bass as _bass

            ncol = (k_pad + 511) // 512  # column-sum segments
            nchunks = len(chunks)
            firstw = chunks[0][1]  # end of the first chunk
            state = {}

            # No HAM warm-up matmuls: the scores pipeline is ACT-paced
            # (~2.6us/q-tile vs cold-PE ~2.2us), so cold matmuls hide
            # behind ACT until the HAM un-throttles on its own.

            def setup_batch_early(b):
                # only what the first QK chunk of the batch needs: shortest
                # possible DMA critical path to the first ACTIVATE.  Keep
                # per-partition runs >= 2KB: narrower column slices degrade
                # into tiny DMA descriptors that throttle the whole queue.
                qt = qtp.tile([128, QL], f16, tag="qt", name=f"qt{b}")
                dt = dtp.tile([128, k_pad], f16, tag="dt", name=f"dt{b}")
                nc.sync.dma_start(qt, qtd.ap()[b])
                nc.sync.dma_start(dt[:, :firstw], dtd.ap()[b][:, :firstw])
                state[b] = (qt, dt)

            def setup_batch_dtrest(b):
                qt, dt = state[b][0], state[b][1]
                if firstw < k_pad:
                    nc.sync.dma_start(
                        dt[:, firstw:], dtd.ap()[b][:, firstw:]
                    )

            def setup_batch_cnt(b):
                # pad count replicated to all 128 partitions via
                # partition-broadcast DMA
                cntk = bsm.tile([128, 1], f32, tag="cntk", name=f"cntk{b}")
                ck_ap = ckd.ap()[b : b + 1, :]
                ck_bcast = _bass.AP(
                    tensor=ck_ap.tensor,
                    offset=ck_ap.offset,
                    ap=[[0, 128], [1, 1]],
                )
                nc.sync.dma_start(cntk, ck_bcast)
                state[b] = (state[b][0], state[b][1], cntk)

            setup_batch_early(0)
            NSTEP = BPC * QT_N
            # column-sums trail the scores pipeline by DEFER steps so the
            # next q-tile's score matmuls never wait on the multiply chain
            DEFER = 1
            pendings = []  # deferred column-sums: (b, t, acc, r128, p_t)

            for s in range(NSTEP + DEFER):
                if s < NSTEP:
                    b, t = divmod(s, QT_N)
                    if t == QT_N // 2 and b + 1 < BPC:
                        setup_batch_early(b + 1)
                        setup_batch_dtrest(b + 1)
                        setup_batch_cnt(b + 1)
                    qt, dt = state[b][0], state[b][1]
                    if t == 0:
                        acc_b = pacc.tile(
                            [128, 512], f32, tag="acc", name=f"acc{b}"
                        )
                        state[(b, "acc")] = acc_b
                    acc = state[(b, "acc")]

                    if s == 0:
                        setup_batch_dtrest(0)
                    sim = simp.tile(
                        [128, k_pad], f16, tag="sim", name=f"sim{b}_{t}"
                    )
                    # alternate the sim streams across the sync HWDGE queue
                    # and the (otherwise idle) GpSimd SWDGE queue for DMA
                    # parallelism
                    dma_eng = nc.sync if t % 2 == 0 else nc.gpsimd
                    dma_eng.dma_start(
                        sim, sd.ap()[b, t * 128 : (t + 1) * 128, :]
                    )
                    if s == 0:
                        # cnt rides behind the first sim stream: its tiny
                        # broadcast must not delay the critical loads
                        setup_batch_cnt(0)
                    e_t = ep.tile([128, k_pad], f16, tag="E", name=f"e{b}_{t}")
                    den2 = small.tile(
                        [128, nchunks], f32, tag="den2", name=f"den2_{b}_{t}"
                    )
                    for ci, (lo, hi, tag, cbufs) in enumerate(chunks):
                        psc = psp.tile(
                            [128, hi - lo], f32, tag=tag, bufs=cbufs,
                            name=f"psc{tag}",
                        )
                        for s0 in range(0, hi - lo, 512):
                            s1 = min(s0 + 512, hi - lo)
                            nc.tensor.matmul(
                                psc[:, s0:s1],
                                qt[:, t * 128 : (t + 1) * 128],
                                dt[:, lo + s0 : lo + s1],
                                start=True,
                                stop=True,
                            )
                        # (the ACT accumulator resets per ACTIVATE, so each
                        # chunk needs its own accum_out slice)
                        nc.scalar.activation(
                            out=e_t[:, lo:hi],
                            in_=psc,
                            func=mybir.ActivationFunctionType.Exp,
                            scale=SCALE,
                            accum_out=den2[:, ci : ci + 1],
                        )

                if s == NSTEP and chunks[0][1] - chunks[0][0] >= 512:
                    # keep-warm: the PE idles ~5us between the last QK and
                    # the last column-sums, long enough for the HAM clock
                    # gate to re-throttle to 1.2GHz.  Two dummy matmuls
                    # gated on the last exp / multiply refresh the activity
                    # window mid-gap so the final column-sums run at 2.4GHz.
                    lo, hi, tag, cbufs = chunks[0]
                    for di, mv in enumerate((last_e, last_p)):
                        pscd = psp.tile(
                            [128, hi - lo], f32, tag=tag, bufs=cbufs,
                            name=f"pscwarm{di}",
                        )
                        nc.tensor.matmul(
                            pscd[:, :512],
                            r128s[1],
                            mv[:, :512],
                            start=True,
                            stop=True,
                            skip_group_check=True,
                        )

                # deferred column-sums: their matmuls land on the PE queue
                # AFTER this step's QK matmuls, so the next q-tiles' scores
                # never wait on the DVE multiply chain or a late sim DMA
                if len(pendings) > (DEFER - 1 if s < NSTEP else -1):
                    pb, pt, pacc_t, pr128, p_prev = pendings.pop(0)
                    for j in range(ncol):
                        s0 = j * 512
                        s1 = min(s0 + 512, k_pad)
                        nc.tensor.matmul(
                            pacc_t[:, : s1 - s0],
                            pr128,
                            p_prev[:, s0:s1],
                            start=(pt == 0 and j == 0),
                            stop=(pt == QT_N - 1 and j == ncol - 1),
                            skip_group_check=True,
                        )
                    if pt == QT_N - 1:
                        # batch epilogue: row 0 of acc = sum_q P[q,k]/den_q
                        nc.vector.reduce_sum(
                            outsb[0:1, pb : pb + 1],
                            pacc_t[0:1, :],
                            axis=mybir.AxisListType.X,
                        )

                if s < NSTEP:
                    cntk = state[b][2]
                    dent = small.tile([128, 1], f32, tag="dent", name="dent")
                    if nchunks == 2:
                        # denA - cnt overlaps chunk B's ACTIVATE; only the
                        # tiny add + reciprocal remain on the tail chain
                        denta = small.tile(
                            [128, 1], f32, tag="denta", name="denta"
                        )
                        nc.vector.tensor_scalar(
                            denta, den2[:, 0:1], cntk, None,
                            mybir.AluOpType.subtract,
                        )
                        nc.vector.tensor_tensor(
                            dent, denta, den2[:, 1:2], mybir.AluOpType.add
                        )
                    else:
                        den = small.tile([128, 1], f32, tag="den", name="den")
                        nc.vector.reduce_sum(
                            den, den2, axis=mybir.AxisListType.X
                        )
                        nc.vector.tensor_scalar(
                            dent, den, cntk, None, mybir.AluOpType.subtract
                        )
                    r128 = r128s[s % 3]
                    with nc.allow_low_precision(
                        reason="1/den in fp16 (11-bit mantissa) feeds the PE "
                        "column-sum; ~5e-4 relative, inside the error budget"
                    ):
                        nc.vector.reciprocal(r128[:, 0:1], dent)
                    # P in two pieces: the first starts right after chunk
                    # A's ACTIVATE instead of waiting for the whole row
                    p_t = pp.tile([128, k_pad], f16, tag="P", name=f"p{b}_{t}")
                    nc.vector.tensor_tensor(
                        p_t[:, :firstw], e_t[:, :firstw], sim[:, :firstw],
                        mybir.AluOpType.mult,
                    )
                    if firstw < k_pad:
                        nc.vector.tensor_tensor(
                            p_t[:, firstw:], e_t[:, firstw:], sim[:, firstw:],
                            mybir.AluOpType.mult,
                        )
                    pendings.append((b, t, acc, r128, p_t))
                    last_e, last_p = e_t, p_t

            nc.sync.dma_start(outd.ap()[:, :], outsb)

    nc.compile()
    return nc


def kernel(**inputs: np.ndarray) -> np.ndarray:
    q = np.asarray(inputs["query_input"], dtype=np.float32)
    d = np.asarray(inputs["doc_input"], dtype=np.float32)
    s = np.asarray(inputs["sim_matrix"], dtype=np.float32)
    dm = np.asarray(inputs["doc_mask"]) != 0  # [B, DL]

    keff = dm.sum(axis=1).astype(np.int64)  # [B]
    k_pad = int(min(DL, max(128, math.ceil(int(keff.max()) / 16) * 16)))

    if k_pad not in _CACHED:
        _CACHED[k_pad] = _build(k_pad)
    nc = _CACHED[k_pad]

    qt = np.ascontiguousarray(np.swapaxes(q, 1, 2)).astype(np.float16)
    dtc = np.zeros((B, H, k_pad), dtype=np.float16)
    simc = np.zeros((B, QL, k_pad), dtype=np.float16)
    for b in range(B):
        idx = np.flatnonzero(dm[b])
        ke = idx.size
        dtc[b, :, :ke] = d[b, idx, :].T
        simc[b, :, :ke] = s[b][:, idx]
    ck = (k_pad - keff).astype(np.float32).reshape(B, 1)

    in_maps = []
    for c in range(NCORES):
        lo, hi = c * BPC, (c + 1) * BPC
        in_maps.append(
            {
                "qt": qt[lo:hi],
                "dt": dtc[lo:hi],
                "s": simc[lo:hi],
                "ck": ck[lo:hi],
            }
        )

    out = None
    for attempt in range(3):
        try:
            res = run_bass_kernel_spmd(nc, in_maps, core_ids=list(range(NCORES)))
            # materialize inside the retry: transient device wedges can
            # surface as late as the device->host copy
            out = np.concatenate(
                [
                    np.asarray(res.results[c]["o"]).reshape(BPC)
                    for c in range(NCORES)
                ],
                axis=0,
            )
            break
        except Exception:
            if attempt == 2:
                raise
    return out.reshape(B, 1).astype(np.float32)


# revision 42
# speedup vs baseline: 1.1286x; 1.1286x over previous
"""Trainium2 Bass kernel for nn_AttenCross (sparse_attention).

reference:
    scores = einsum('bqd,bkd->bqk', Q, D) / sqrt(H)
    scores = where(doc_mask==0, -9999, scores)
    attn   = softmax(scores, -1)
    out    = sum over k of (attn * sim), then sum over q -> (B, 1)

Strategy (8 cores, data-parallel over batch, 2 batches/core), v2:

Host-side prep (sharding/layout/encoding only, exact for any inputs):
~50% of doc positions are masked (doc_mask ~ Bernoulli(0.5)), and masked
columns contribute exactly nothing once handled by counting, so the doc
axis is COMPACTED host-side: keep only unmasked doc columns (of D and
sim), zero-pad to K_pad = ceil(max_b keff[b]/128)*128.  A padded column
of D is all-zero => its score is exactly 0 => exp is exactly 1, so
subtracting the pad count from the exp row-sum reproduces the exact
softmax denominator; padded sim columns are zero so they add nothing to
the numerator.  (No row-max subtraction: scores ~ N(0,1); softmax is
shift-invariant.)  All tensors are converted to fp16 host-side: PE runs
fp16 at full rate (1 col/cycle vs ~2.2 for f32r), DMA bytes halve, and
DVE 16-bit ops run in 2x/4x perf modes; fp16's 10-bit mantissa keeps the
overall rel-err ~1e-3.

Device, per batch (per q-tile of 128 queries), ACT(exp)-paced pipeline:
  - PE: QK^T fp16 matmuls into PSUM chunks (1024 double-buffered +
    K_pad-1024 single-buffered; PSUM is 8 banks: 4 + 3 + 1 for the
    column-sum accumulator -- a full-width double-buffered score tile
    would not fit, which is what forces 2 ACTIVATEs per q-tile).
  - ACT: E = exp(scale*psum) -> fp16 SBUF, one ACTIVATE per chunk, with
    accum_out producing the per-chunk row-sums (den parts) for free-ish
    (the DVE alternatives -- tensor_scalar-with-accum aka
    TENSOR_SCALAR_CACHE_REDUCE, and tensor_reduce -- both run at 1x
    ~2.5us per q-tile on HW; ACT's ACTIVATION_READ_ACCUMULATOR is 311ns;
    the ACT accumulator resets per ACTIVATE so each chunk needs its own).
  - DVE: den-parts sum + cnt subtract + reciprocal -> w into column 0 of
    a rotating fp16 [128,128] tile; P = E * sim fp16 (2x mode), split at
    the chunk boundary so it starts right after chunk A's exp.
  - PE: column-sum matmuls with w as the stationary operand accumulate
    sum_q P[q,k]/den_q into a per-batch [128,512] PSUM bank; row 0 of
    that bank reduced (DVE) to the scalar batch output at the end.
    These trail the scores pipeline by one q-tile (software pipelining)
    so the next q-tile's QK matmuls issue ahead of them on the PE queue.
  - DMA: sim streams alternate between the sync HWDGE queue and the
    GpSimd SWDGE queue; per-partition runs are kept >= 2KB (narrow
    column slices degrade into tiny descriptors that throttle the
    queue); no HAM warm-up (cold PE hides behind the ACT pacing).
Output per core: [1, BPC] fp32; host stacks to [16, 1] fp32.

Measured on the target HW: ~68-70us (vs 138us for the f32r/full-K
baseline); exp streaming is the bound: 16 q-tiles x (2128 elems + 2
instruction bubbles + 2 accumulator reads)/1.2GHz ~= 44us, plus ~7us
framework preamble, ~4us fill, ~4us drain chain, ~9.5us teardown.
"""

import math

import numpy as np

import concourse.bacc as bacc
import concourse.tile as tile
import concourse.mybir as mybir
from concourse.bass_utils import run_bass_kernel_spmd

B, QL, DL, H = 16, 1024, 4096, 128
NCORES = 8
BPC = B // NCORES  # batches per core
QT_N = QL // 128  # 8 q-tiles per batch
SCALE = 1.0 / float(np.sqrt(H))

f32 = mybir.dt.float32
f16 = mybir.dt.float16

_CACHED = {}


def _plan_chunks(k_pad):
    """Split the doc axis into PSUM-resident chunks: (offset_lo, offset_hi,
    tag, bufs).

    Expected path (k_pad <= 2560): chunk A [128,1024] double-buffered (4
    banks) + chunk B [128, k_pad-1024 <= 1536] single-buffered (<=3
    banks) + 1 bank for the column-sum accumulator = 8.  Generic
    fallback for larger k_pad: 1024-wide chunks cycling three
    single-buffered tags."""
    if k_pad <= 1024:
        return [(0, k_pad, "A", 2)]
    if k_pad <= 2560:
        return [(0, 1024, "A", 2), (1024, k_pad, "B", 1)]
    chunks = []
    off = 0
    i = 0
    while off < k_pad:
        w = min(1024, k_pad - off)
        chunks.append((off, off + w, "BCD"[i % 3], 1))
        off += w
        i += 1
    return chunks


def _build(k_pad):
    chunks = _plan_chunks(k_pad)

    nc = bacc.Bacc("TRN2", target_bir_lowering=False, debug=False)

    qtd = nc.dram_tensor("qt", [BPC, H, QL], f16, kind="ExternalInput")
    dtd = nc.dram_tensor("dt", [BPC, H, k_pad], f16, kind="ExternalInput")
    sd = nc.dram_tensor("s", [BPC, QL, k_pad], f16, kind="ExternalInput")
    ckd = nc.dram_tensor("ck", [BPC, 1], f32, kind="ExternalInput")
    outd = nc.dram_tensor("o", [1, BPC], f32, kind="ExternalOutput")

    with tile.TileContext(nc) as tc:
        with (
            tc.tile_pool(name="const", bufs=1) as const,
            tc.tile_pool(name="qtp", bufs=2) as qtp,
            tc.tile_pool(name="dtp", bufs=2) as dtp,
            tc.tile_pool(name="simp", bufs=4) as simp,
            tc.tile_pool(name="ep", bufs=3) as ep,
            tc.tile_pool(name="pp", bufs=3) as pp,
            tc.tile_pool(name="small", bufs=4) as small,
            tc.tile_pool(name="bsm", bufs=2) as bsm,
            tc.tile_pool(name="outp", bufs=1) as outp,
            tc.tile_pool(name="ps", bufs=1, space="PSUM") as psp,
            tc.tile_pool(name="pacc", bufs=1, space="PSUM") as pacc,
        ):
            # stationary w-tiles for the column-sum matmuls: col 0 = 1/den,
            # cols 1..127 stay zero forever (three rotating, since the
            # column-sums trail the scores pipeline by two steps)
            r128s = []
            for ri in range(3):
                r = const.tile([128, 128], f16, tag=f"r128_{ri}", name=f"r128_{ri}")
                nc.vector.memset(r, 0.0)
                r128s.append(r)

            outsb = outp.tile([1, BPC], f32, tag="outsb")

            import concourse.bass as _bass

            ncol = (k_pad + 511) // 512  # column-sum segments
            nchunks = len(chunks)
            firstw = chunks[0][1]  # end of the first chunk
            state = {}

            # Light HAM warm-up: 32 narrow (128-wide) matmuls ~= 3.4us of
            # cold-PE activity that exactly fills the initial DMA wait, so
            # the first q-tiles' score matmuls run at 2.4GHz instead of
            # 1.2GHz (the early ACT gaps wait on exactly those).  Wide
            # warm-ups (16x512) are counterproductive -- ~11us of cold PE
            # queued ahead of the first real matmul.
            warm = pacc.tile([128, 512], f32, tag="acc", name="warm")
            for _ in range(32):
                nc.tensor.matmul(
                    warm[:, :128], r128s[2], r128s[2],
                    start=True, stop=True, skip_group_check=True,
                )

            def setup_batch_early(b):
                # only what the first QK chunk of the batch needs: shortest
                # possible DMA critical path to the first ACTIVATE.  Keep
                # per-partition runs >= 2KB: narrower column slices degrade
                # into tiny DMA descriptors that throttle the whole queue.
                qt = qtp.tile([128, QL], f16, tag="qt", name=f"qt{b}")
                dt = dtp.tile([128, k_pad], f16, tag="dt", name=f"dt{b}")
                nc.sync.dma_start(qt, qtd.ap()[b])
                nc.sync.dma_start(dt[:, :firstw], dtd.ap()[b][:, :firstw])
                state[b] = (qt, dt)

            def setup_batch_dtrest(b):
                qt, dt = state[b][0], state[b][1]
                if firstw < k_pad:
                    nc.sync.dma_start(
                        dt[:, firstw:], dtd.ap()[b][:, firstw:]
                    )

            def setup_batch_cnt(b):
                # pad count replicated to all 128 partitions via
                # partition-broadcast DMA
                cntk = bsm.tile([128, 1], f32, tag="cntk", name=f"cntk{b}")
                ck_ap = ckd.ap()[b : b + 1, :]
                ck_bcast = _bass.AP(
                    tensor=ck_ap.tensor,
                    offset=ck_ap.offset,
                    ap=[[0, 128], [1, 1]],
                )
                nc.sync.dma_start(cntk, ck_bcast)
                state[b] = (state[b][0], state[b][1], cntk)

            setup_batch_early(0)
            NSTEP = BPC * QT_N
            # column-sums trail the scores pipeline by DEFER steps so the
            # next q-tile's score matmuls never wait on the multiply chain
            DEFER = 1
            pendings = []  # deferred column-sums: (b, t, acc, r128, p_t)

            for s in range(NSTEP + DEFER):
                if s < NSTEP:
                    b, t = divmod(s, QT_N)
                    if t == QT_N // 2 and b + 1 < BPC:
                        setup_batch_early(b + 1)
                        setup_batch_dtrest(b + 1)
                        setup_batch_cnt(b + 1)
                    qt, dt = state[b][0], state[b][1]
                    if t == 0:
                        acc_b = pacc.tile(
                            [128, 512], f32, tag="acc", name=f"acc{b}"
                        )
                        state[(b, "acc")] = acc_b
                    acc = state[(b, "acc")]

                    if s == 0:
                        setup_batch_dtrest(0)
                    sim = simp.tile(
                        [128, k_pad], f16, tag="sim", name=f"sim{b}_{t}"
                    )
                    # alternate the sim streams across the sync HWDGE queue
                    # and the (otherwise idle) GpSimd SWDGE queue for DMA
                    # parallelism
                    dma_eng = nc.sync if t % 2 == 0 else nc.gpsimd
                    dma_eng.dma_start(
                        sim, sd.ap()[b, t * 128 : (t + 1) * 128, :]
                    )
                    if s == 0:
                        # cnt rides behind the first sim stream: its tiny
                        # broadcast must not delay the critical loads
                        setup_batch_cnt(0)
                    e_t = ep.tile([128, k_pad], f16, tag="E", name=f"e{b}_{t}")
                    den2 = small.tile(
                        [128, nchunks], f32, tag="den2", name=f"den2_{b}_{t}"
                    )
                    for ci, (lo, hi, tag, cbufs) in enumerate(chunks):
                        psc = psp.tile(
                            [128, hi - lo], f32, tag=tag, bufs=cbufs,
                            name=f"psc{tag}",
                        )
                        for s0 in range(0, hi - lo, 512):
                            s1 = min(s0 + 512, hi - lo)
                            nc.tensor.matmul(
                                psc[:, s0:s1],
                                qt[:, t * 128 : (t + 1) * 128],
                                dt[:, lo + s0 : lo + s1],
                                start=True,
                                stop=True,
                            )
                        # (the ACT accumulator resets per ACTIVATE, so each
                        # chunk needs its own accum_out slice)
                        nc.scalar.activation(
                            out=e_t[:, lo:hi],
                            in_=psc,
                            func=mybir.ActivationFunctionType.Exp,
                            scale=SCALE,
                            accum_out=den2[:, ci : ci + 1],
                        )

                if s == NSTEP and chunks[0][1] - chunks[0][0] >= 512:
                    # keep-warm: the PE idles ~5us between the last QK and
                    # the last column-sums, long enough for the HAM clock
                    # gate to re-throttle to 1.2GHz.  Two dummy matmuls
                    # gated on the last exp / multiply refresh the activity
                    # window mid-gap so the final column-sums run at 2.4GHz.
                    lo, hi, tag, cbufs = chunks[0]
                    for di, mv in enumerate((last_e, last_p)):
                        pscd = psp.tile(
                            [128, hi - lo], f32, tag=tag, bufs=cbufs,
                            name=f"pscwarm{di}",
                        )
                        nc.tensor.matmul(
                            pscd[:, :512],
                            r128s[1],
                            mv[:, :512],
                            start=True,
                            stop=True,
                            skip_group_check=True,
                        )

                # deferred column-sums: their matmuls land on the PE queue
                # AFTER this step's QK matmuls, so the next q-tiles' scores
                # never wait on the DVE multiply chain or a late sim DMA
                if len(pendings) > (DEFER - 1 if s < NSTEP else -1):
                    pb, pt, pacc_t, pr128, p_prev = pendings.pop(0)
                    for j in range(ncol):
                        s0 = j * 512
                        s1 = min(s0 + 512, k_pad)
                        nc.tensor.matmul(
                            pacc_t[:, : s1 - s0],
                            pr128,
                            p_prev[:, s0:s1],
                            start=(pt == 0 and j == 0),
                            stop=(pt == QT_N - 1 and j == ncol - 1),
                            skip_group_check=True,
                        )
                    if pt == QT_N - 1:
                        # batch epilogue: row 0 of acc = sum_q P[q,k]/den_q
                        nc.vector.reduce_sum(
                            outsb[0:1, pb : pb + 1],
                            pacc_t[0:1, :],
                            axis=mybir.AxisListType.X,
                        )

                if s < NSTEP:
                    cntk = state[b][2]
                    dent = small.tile([128, 1], f32, tag="dent", name="dent")
                    if nchunks == 2:
                        # denA - cnt overlaps chunk B's ACTIVATE; only the
                        # tiny add + reciprocal remain on the tail chain
                        denta = small.tile(
                            [128, 1], f32, tag="denta", name="denta"
                        )
                        nc.vector.tensor_scalar(
                            denta, den2[:, 0:1], cntk, None,
                            mybir.AluOpType.subtract,
                        )
                        nc.vector.tensor_tensor(
                            dent, denta, den2[:, 1:2], mybir.AluOpType.add
                        )
                    else:
                        den = small.tile([128, 1], f32, tag="den", name="den")
                        nc.vector.reduce_sum(
                            den, den2, axis=mybir.AxisListType.X
                        )
                        nc.vector.tensor_scalar(
                            dent, den, cntk, None, mybir.AluOpType.subtract
                        )
                    r128 = r128s[s % 3]
                    with nc.allow_low_precision(
                        reason="1/den in fp16 (11-bit mantissa) feeds the PE "
                        "column-sum; ~5e-4 relative, inside the error budget"
                    ):
                        nc.vector.reciprocal(r128[:, 0:1], dent)
                    # P in two pieces: the first starts right after chunk
                    # A's ACTIVATE instead of waiting for the whole row
                    p_t = pp.tile([128, k_pad], f16, tag="P", name=f"p{b}_{t}")
                    nc.vector.tensor_tensor(
                        p_t[:, :firstw], e_t[:, :firstw], sim[:, :firstw],
                        mybir.AluOpType.mult,
                    )
                    if firstw < k_pad:
                        nc.vector.tensor_tensor(
                            p_t[:, firstw:], e_t[:, firstw:], sim[:, firstw:],
                            mybir.AluOpType.mult,
                        )
                    pendings.append((b, t, acc, r128, p_t))
                    last_e, last_p = e_t, p_t

            nc.sync.dma_start(outd.ap()[:, :], outsb)

    nc.compile()
    return nc


def kernel(**inputs: np.ndarray) -> np.ndarray:
    q = np.asarray(inputs["query_input"], dtype=np.float32)
    d = np.asarray(inputs["doc_input"], dtype=np.float32)
    s = np.asarray(inputs["sim_matrix"], dtype=np.float32)
    dm = np.asarray(inputs["doc_mask"]) != 0  # [B, DL]

    keff = dm.sum(axis=1).astype(np.int64)  # [B]
    k_pad = int(min(DL, max(128, math.ceil(int(keff.max()) / 16) * 16)))

    if k_pad not in _CACHED:
        _CACHED[k_pad] = _build(k_pad)
    nc = _CACHED[k_pad]

    qt = np.ascontiguousarray(np.swapaxes(q, 1, 2)).astype(np.float16)
    dtc = np.zeros((B, H, k_pad), dtype=np.float16)
    simc = np.zeros((B, QL, k_pad), dtype=np.float16)
    for b in range(B):
        idx = np.flatnonzero(dm[b])
        ke = idx.size
        dtc[b, :, :ke] = d[b, idx, :].T
        simc[b, :, :ke] = s[b][:, idx]
    ck = (k_pad - keff).astype(np.float32).reshape(B, 1)

    in_maps = []
    for c in range(NCORES):
        lo, hi = c * BPC, (c + 1) * BPC
        in_maps.append(
            {
                "qt": qt[lo:hi],
                "dt": dtc[lo:hi],
                "s": simc[lo:hi],
                "ck": ck[lo:hi],
            }
        )

    out = None
    for attempt in range(3):
        try:
            res = run_bass_kernel_spmd(nc, in_maps, core_ids=list(range(NCORES)))
            # materialize inside the retry: transient device wedges can
            # surface as late as the device->host copy
            out = np.concatenate(
                [
                    np.asarray(res.results[c]["o"]).reshape(BPC)
                    for c in range(NCORES)
                ],
                axis=0,
            )
            break
        except Exception:
            if attempt == 2:
                raise
    return out.reshape(B, 1).astype(np.float32)
